# revision 1
# baseline (speedup 1.0000x reference)
"""Trainium2 Bass kernel for sparse_attention nn_CWAB_34050500722860.

Model (per batch b, S=4096 tokens, D=1024, H=16 heads of 64):
  xp = x + pos_emb[:S]
  local: non-overlapping 512-token window self-attention per head
  global: keys = concat(conv4-compressed first-256-tokens (G=64), global_memory (64))
  out = sigmoid([local||glob] @ gate_w + gate_b) gated mix, then @ out_w + out_b

Sharding: 8 cores = 4 batches x 2 sequence halves (2048 tokens = 4 windows
per core). Every core computes its batch's 128 global tokens from a
replicated copy of the first 256 tokens, so there are no collectives.

Dataflow (per core, all matmuls on PE):
  - scores are computed TRANSPOSED: sT[k,q] = lhsT(xT k-slice)^T @ rhs(xT q)
    so softmax normalization runs over the PSUM partition dim and the
    attention output comes out feature-major (localT[d,q]) - exactly the
    layout the gate matmul needs as lhsT. No probability transposes.
  - V carries an appended ones column, so the PV matmul's row 64 is the
    softmax denominator (colsum) for free; normalization happens in the
    PSUM->SBUF evacuation multiply.
  - max-subtraction is skipped: scores*scale is ~N(0,1) with worst case
    ~14 (self-attention diagonal), exp() stays far inside fp32 range.
  - attention path float32r (TF32-class, 1 PE cycle/row); gate/out/conv
    matmuls bf16. PSUM accumulation is always fp32.
"""

import os
from contextlib import ExitStack

import numpy as np
import ml_dtypes

import jax
import jax.numpy as jnp
from jax.sharding import Mesh, PartitionSpec
from jax.experimental.shard_map import shard_map

import bass_rust
import concourse.bass as bass
import concourse.mybir as mybir
import concourse.tile as tile
from concourse import bass2jax
from concourse.masks import make_identity
from concourse.vector_clock import ScopedClock

F32 = mybir.dt.float32
F32R = mybir.dt.float32r
BF16 = mybir.dt.bfloat16
BF16_NP = ml_dtypes.bfloat16

N_CORES = 8
B, S, D = 4, 4096, 1024
H, HD, WIN, G = 16, 64, 512, 64
SC = S // 2          # tokens per core
NW = SC // WIN       # 4 windows per core
SCALE = 1.0 / np.sqrt(HD)
ACT_EXP = mybir.ActivationFunctionType.Exp
ACT_SIG = mybir.ActivationFunctionType.Sigmoid


class SplitDrainTileContext(tile.TileContext):
    """Walrus in this container rejects a Drain carrying >2 sem waits.

    Tile's exit puts a wait for every engine/queue sem on one Drain; strip
    them and re-emit as single-wait NOPs on the sync sequencer (sequencer
    order keeps the all-engine barrier behind every wait)."""

    def _drain_and_barrier(self, tick_clock, wait_clock):
        nc = self.nc
        drain_inst = nc.sync.drain()
        wait_clock.add_sem_waits(
            drain_inst.ins, ScopedClock({None: tick_clock.global_clock})
        )
        si = drain_inst.ins.sync_info
        waits = list(si.on_wait) if si is not None and si.on_wait else []
        if len(waits) > 1:
            updates = list(si.on_update) if si is not None and si.on_update else []
            drain_inst.ins.sync_info = bass_rust.SyncInfo(
                on_wait=waits[:1], on_update=updates
            )
            for w in waits[1:]:
                nop = nc.sync.nop()
                nop.ins.sync_info = bass_rust.SyncInfo(on_wait=[w], on_update=[])

        nc.all_engine_barrier()
        assert self.sems is not None
        popped = nc._tile_sem_poison_stack.pop()
        assert popped is self._sem_poison
        nc.clear_and_free_semaphores(list(self.sems.allocated().values()))
        nc.all_engine_barrier()


def split_sync_waits(nc, limit: int = 1):
    """This walrus build accepts at most one sem wait per instruction.

    For any instruction carrying more, peel the extras onto same-engine
    NoOps inserted directly before it (engine streams preserve bb order,
    so the engine still blocks on every wait before executing it)."""
    for fn in nc.m.functions:
        for bb in fn.blocks:
            il = bb.instructions
            out, changed, k = [], False, 0
            for inst in il:
                si = inst.sync_info
                waits = list(si.on_wait) if si is not None and si.on_wait else []
                if len(waits) > limit and inst.engine != mybir.EngineType.Unassigned:
                    for w in waits[:-limit]:
                        nop = mybir.InstNoOp(
                            name=f"{inst.name}-wsplit{k}", engine=inst.engine
                        )
                        nop.sync_info = bass_rust.SyncInfo(on_wait=[w], on_update=[])
                        out.append(nop)
                        k += 1
                    inst.sync_info = bass_rust.SyncInfo(
                        on_wait=waits[-limit:],
                        on_update=list(si.on_update) if si.on_update else [],
                    )
                    changed = True
                out.append(inst)
            if changed:
                bb.instructions = out


def build_nc(nrep: int = 1, dbg: bool = False):
    abl = os.environ.get("KERNEL_ABL", "")
    nc = bass.Bass("TRN2", target_bir_lowering=False, debug=False)

    x_d = nc.declare_dram_parameter("x", [SC, D], F32, isOutput=False).ap()
    pos_d = nc.declare_dram_parameter("pos", [SC, D], F32, isOutput=False).ap()
    xg_d = nc.declare_dram_parameter("xg", [4 * G, D], F32, isOutput=False).ap()
    posg_d = nc.declare_dram_parameter("posg", [4 * G, D], F32, isOutput=False).ap()
    gmem_d = nc.declare_dram_parameter("gmem", [G, D], F32, isOutput=False).ap()
    cw_d = nc.declare_dram_parameter("cw", [4 * D, D], BF16, isOutput=False).ap()
    cb_d = nc.declare_dram_parameter("cb", [1, D], F32, isOutput=False).ap()
    gw_d = nc.declare_dram_parameter("gw", [2 * D, D], BF16, isOutput=False).ap()
    gb_d = nc.declare_dram_parameter("gb", [8, 128], F32, isOutput=False).ap()
    ow_d = nc.declare_dram_parameter("ow", [D, D], BF16, isOutput=False).ap()
    ob_d = nc.declare_dram_parameter("ob", [1, D], F32, isOutput=False).ap()
    out_d = nc.declare_dram_parameter("out", [SC, D], F32, isOutput=True).ap()

    x_r = x_d.rearrange("(n p) d -> n p d", p=128)       # [16,128,1024]
    pos_r = pos_d.rearrange("(n p) d -> n p d", p=128)
    xg_r = xg_d.rearrange("(n p) d -> n p d", p=128)     # [2,128,1024]
    posg_r = posg_d.rearrange("(n p) d -> n p d", p=128)
    out_r = out_d.rearrange("(n p) d -> n p d", p=128)

    with SplitDrainTileContext(nc) as tc, ExitStack() as ctx:
        const = ctx.enter_context(tc.tile_pool(name="const", bufs=1))
        wpool = ctx.enter_context(tc.tile_pool(name="weights", bufs=1))
        cwp = ctx.enter_context(tc.tile_pool(name="cw_stream", bufs=5))
        stage = ctx.enter_context(tc.tile_pool(name="stage", bufs=2))
        xwp = ctx.enter_context(tc.tile_pool(name="xw", bufs=2))
        xtp = ctx.enter_context(tc.tile_pool(name="xt", bufs=1))
        expp = ctx.enter_context(tc.tile_pool(name="expt", bufs=4))
        rowp = ctx.enter_context(tc.tile_pool(name="rows", bufs=2))
        ltp = ctx.enter_context(tc.tile_pool(name="localT", bufs=2))
        gtp = ctx.enter_context(tc.tile_pool(name="globT", bufs=2))
        mixp = ctx.enter_context(tc.tile_pool(name="mix", bufs=1))
        outp = ctx.enter_context(tc.tile_pool(name="ostage", bufs=1))
        rbp = ctx.enter_context(tc.tile_pool(name="rb", bufs=2))
        gpool = ctx.enter_context(tc.tile_pool(name="gt", bufs=1))

        ps_tr = ctx.enter_context(tc.tile_pool(name="ps_tr", bufs=1, space="PSUM"))
        ps_st = ctx.enter_context(tc.tile_pool(name="ps_st", bufs=2, space="PSUM"))
        ps_pv = ctx.enter_context(tc.tile_pool(name="ps_pv", bufs=2, space="PSUM"))
        ps_gate = ctx.enter_context(tc.tile_pool(name="ps_gate", bufs=1, space="PSUM"))
        ps_big = ctx.enter_context(tc.tile_pool(name="ps_big", bufs=1, space="PSUM"))

        # ---- constants / persistent weights (outside the repeat loop) ----
        ident_f = const.tile([128, 128], F32)
        make_identity(nc, ident_f)
        ident_r = const.tile([128, 128], F32R)
        nc.vector.tensor_copy(out=ident_r, in_=ident_f)
        ident_b = const.tile([128, 128], BF16)
        nc.vector.tensor_copy(out=ident_b, in_=ident_f)
        ones_f = const.tile([128, 64], F32)
        nc.vector.memset(ones_f, 1.0)
        # selectors (K=1 lhsT rows): top covers out partitions 0-63, bottom
        # 64-127; two accumulating matmuls replicate a head-pair's reciprocal
        # rows into one [128, q] psum bank.
        selt_f = const.tile([1, 128], F32)
        nc.vector.memset(selt_f, 0.0)
        nc.vector.memset(selt_f[0:1, 0:64], 1.0)
        selb_f = const.tile([1, 128], F32)
        nc.vector.memset(selb_f, 0.0)
        nc.vector.memset(selb_f[0:1, 64:128], 1.0)
        sel_top = const.tile([1, 128], F32R)
        nc.vector.tensor_copy(out=sel_top, in_=selt_f)
        sel_bot = const.tile([1, 128], F32R)
        nc.vector.tensor_copy(out=sel_bot, in_=selb_f)

        gw_sb = wpool.tile([128, 16, D], BF16)
        nc.sync.dma_start(out=gw_sb, in_=gw_d.rearrange("(k p) n -> p k n", p=128))
        ow_sb = wpool.tile([128, 8, D], BF16)
        nc.sync.dma_start(out=ow_sb, in_=ow_d.rearrange("(k p) n -> p k n", p=128))
        gb_sb = wpool.tile([128, 8], F32)
        nc.sync.dma_start(out=gb_sb, in_=gb_d.rearrange("m p -> p m"))
        obb = wpool.tile([128, D], F32)
        nc.sync.dma_start(out=obb, in_=ob_d.to_broadcast([128, D]))
        cbb = wpool.tile([G, D], F32)
        nc.sync.dma_start(out=cbb, in_=cb_d.to_broadcast([G, D]))
        gm_sb = wpool.tile([G, D], F32)
        nc.sync.dma_start(out=gm_sb, in_=gmem_d)

        if dbg:
            dbg_l = nc.declare_dram_parameter(
                "dbg_l", [NW, D, WIN], F32, isOutput=True
            ).ap()
            dbg_g = nc.declare_dram_parameter(
                "dbg_g", [NW, D, WIN], F32, isOutput=True
            ).ap()
            dbg_gt = nc.declare_dram_parameter(
                "dbg_gt", [128, H, HD + 1], F32, isOutput=True
            ).ap()

        def normalize_pair(pvA, pvB, outA, outB):
            """softmax denominators sit in row HD of pvA/pvB; write the
            normalized [64, 512] blocks to outA/outB (bf16 SBUF)."""
            rrA = rowp.tile([1, WIN], F32R, tag="rr")
            with nc.allow_low_precision(reason="softmax denom"):
                nc.vector.reciprocal(out=rrA, in_=pvA[HD : HD + 1, :])
            rrB = rowp.tile([1, WIN], F32R, tag="rr")
            with nc.allow_low_precision(reason="softmax denom"):
                nc.vector.reciprocal(out=rrB, in_=pvB[HD : HD + 1, :])
            rcb = ps_st.tile([128, WIN], F32, tag="st")
            nc.tensor.matmul(rcb, sel_top, rrA, start=True, stop=False)
            nc.tensor.matmul(rcb, sel_bot, rrB, start=False, stop=True)
            rb = rbp.tile([128, WIN], F32, tag="rb")
            nc.scalar.copy(out=rb, in_=rcb)
            nc.vector.tensor_mul(out=outA, in0=pvA[0:HD, :], in1=rb[0:64, :])
            nc.vector.tensor_mul(out=outB, in0=pvB[0:HD, :], in1=rb[64:128, :])

        def emit_body():
            # ================= conv -> gt (global tokens) =================
            # xg_sum = bf16(xg + posg), token-major [128, 2, 1024]
            xg_sum = gpool.tile([128, 2, D], BF16, tag="xgsum")
            for tt in range(2):
                for hh in range(2):
                    xs = stage.tile([128, 512], F32, tag="xs")
                    nc.sync.dma_start(
                        out=xs, in_=xg_r[tt][:, hh * 512 : (hh + 1) * 512]
                    )
                    ps = stage.tile([128, 512], F32, tag="ps")
                    nc.sync.dma_start(
                        out=ps, in_=posg_r[tt][:, hh * 512 : (hh + 1) * 512]
                    )
                    nc.gpsimd.tensor_add(
                        out=xg_sum[:, tt, hh * 512 : (hh + 1) * 512], in0=xs, in1=ps
                    )

            # xgp[p, k, ib, g] = xg_sum[4g+k, ib*128+p]  (bf16 lhsT for conv)
            xgp = gpool.tile([128, 4, 8, G], BF16, tag="xgp")
            for ib in range(8):
                pt = ps_tr.tile([128, 2, 128], BF16, tag="trx")
                for tt in range(2):
                    nc.tensor.transpose(
                        pt[:, tt, :], xg_sum[:, tt, ib * 128 : (ib + 1) * 128], ident_b
                    )
                ptv = pt.rearrange("p a b -> p (a b)")  # [128, 256] = [i, t]
                for k in range(4):
                    nc.vector.tensor_copy(
                        out=xgp[:, k, ib, :], in_=ptv[:, k :: 4]
                    )

            # comp[g, o] = sum_f xgT[f, g] cw[f, o] + cb  (both halves in one
            # pass; the gate/out psum banks are free until window-0 gating)
            gt_tok = gpool.tile([128, H, HD + 1], F32R, tag="gt_tok")
            comp0 = ps_gate.tile([G, 512], F32, tag="gate")
            comp1 = ps_big.tile([G, 512], F32, tag="big")
            for kt in range(32):
                cwc = cwp.tile([128, D], BF16, tag="cw")
                nc.sync.dma_start(
                    out=cwc, in_=cw_d.rearrange("(k p) n -> k p n", p=128)[kt]
                )
                k, ib = kt // 8, kt % 8
                nc.tensor.matmul(
                    comp0, xgp[:, k, ib, :], cwc[:, 0:512],
                    start=(kt == 0), stop=(kt == 31),
                )
                nc.tensor.matmul(
                    comp1, xgp[:, k, ib, :], cwc[:, 512:1024],
                    start=(kt == 0), stop=(kt == 31),
                )
            for nt, cps in ((0, comp0), (1, comp1)):
                nc.vector.tensor_add(
                    out=gt_tok[0:G, nt * 8 : (nt + 1) * 8, 0:HD],
                    in0=cps.rearrange("p (h d) -> p h d", h=8),
                    in1=cbb[:, nt * 512 : (nt + 1) * 512].rearrange(
                        "p (h d) -> p h d", h=8
                    ),
                )
            nc.vector.tensor_copy(
                out=gt_tok[G:128, :, 0:HD],
                in_=gm_sb.rearrange("g (h d) -> g h d", h=H),
            )
            nc.vector.tensor_copy(out=gt_tok[:, :, HD], in_=ones_f[:, 0:H])

            # gtT[p, hp, g] = gt[g, hp*128+p]
            gtT = gpool.tile([128, 8, 128], F32R, tag="gtT")
            for hp in range(8):
                for sub in range(2):
                    pt = ps_tr.tile([64, 4, 128], F32R, tag="tr")
                    nc.tensor.transpose(
                        pt[:, 0, :], gt_tok[:, 2 * hp + sub, 0:HD], ident_r
                    )
                    nc.vector.tensor_copy(
                        out=gtT[sub * 64 : sub * 64 + 64, hp, :], in_=pt[:, 0, :]
                    )

            if dbg:
                nc.gpsimd.dma_start(out=dbg_gt, in_=gt_tok)

            # ========================= windows =========================
            for w in range(NW):
                # xw[p, kt, h, 0:64] = f32r(x+pos), col 64 = 1  (V_aug, PV lhsT)
                xw = xwp.tile([128, 4, H, HD + 1], F32R, tag="xw")
                for kt in range(4):
                    for hh in range(2):
                        xs = stage.tile([128, 512], F32, tag="xs")
                        nc.sync.dma_start(
                            out=xs, in_=x_r[w * 4 + kt][:, hh * 512 : (hh + 1) * 512]
                        )
                        ps = stage.tile([128, 512], F32, tag="ps")
                        nc.sync.dma_start(
                            out=ps, in_=pos_r[w * 4 + kt][:, hh * 512 : (hh + 1) * 512]
                        )
                        nc.gpsimd.tensor_add(
                            out=xw[:, kt, hh * 8 : (hh + 1) * 8, 0:HD],
                            in0=xs.rearrange("p (h d) -> p h d", h=8),
                            in1=ps.rearrange("p (h d) -> p h d", h=8),
                        )
                nc.gpsimd.tensor_copy(
                    out=xw[:, :, :, HD],
                    in_=ones_f.rearrange("p (a b) -> p a b", a=4),
                )

                # xT[p, hp, q] = xp[w*512+q, hp*128+p]
                xT = xtp.tile([128, 8, WIN], F32R, tag="xT")
                for hp in range(8):
                    for sub in range(2):
                        pt = ps_tr.tile([64, 4, 128], F32R, tag="tr")
                        for kt in range(4):
                            nc.tensor.transpose(
                                pt[:, kt, :], xw[:, kt, 2 * hp + sub, 0:HD], ident_r
                            )
                        nc.vector.tensor_copy(
                            out=xT[sub * 64 : sub * 64 + 64, hp, :],
                            in_=pt.rearrange("p a b -> p (a b)"),
                        )

                localT = ltp.tile([128, 8, WIN], BF16, tag="localT")
                globT = gtp.tile([128, 8, WIN], BF16, tag="globT")

                if abl == "noattn":
                    for db in range(8):
                        nc.gpsimd.tensor_copy(
                            out=localT[:, db, :], in_=xT[:, db, :]
                        )
                        nc.gpsimd.tensor_copy(
                            out=globT[:, db, :], in_=xT[:, db, :]
                        )
                for hp in range(8 if abl != "noattn" else 0):
                    xTA = xT[0:64, hp, :]      # head 2hp   [64, 512]
                    xTB = xT[64:128, hp, :]    # head 2hp+1 [64, 512]

                    # ---- local pair: sT, exp, PV(ones-col), normalize ----
                    pvA = ps_pv.tile([HD + 1, WIN], F32, tag="pv")
                    pvB = ps_pv.tile([HD + 1, WIN], F32, tag="pv")
                    for kt in range(4):
                        stA = ps_st.tile([128, WIN], F32, tag="st")
                        nc.tensor.matmul(
                            stA, xTA[:, kt * 128 : (kt + 1) * 128], xTA,
                            start=True, stop=True,
                        )
                        stB = ps_st.tile([128, WIN], F32, tag="st")
                        nc.tensor.matmul(
                            stB, xTB[:, kt * 128 : (kt + 1) * 128], xTB,
                            start=True, stop=True,
                        )
                        etA = expp.tile([128, WIN], F32R, tag="et")
                        nc.scalar.activation(
                            out=etA, in_=stA, func=ACT_EXP, scale=SCALE
                        )
                        etB = expp.tile([128, WIN], F32R, tag="et")
                        nc.scalar.activation(
                            out=etB, in_=stB, func=ACT_EXP, scale=SCALE
                        )
                        nc.tensor.matmul(
                            pvA, xw[:, kt, 2 * hp, :], etA,
                            start=(kt == 0), stop=(kt == 3),
                        )
                        nc.tensor.matmul(
                            pvB, xw[:, kt, 2 * hp + 1, :], etB,
                            start=(kt == 0), stop=(kt == 3),
                        )
                    normalize_pair(
                        pvA, pvB,
                        localT[0:64, hp, :], localT[64:128, hp, :],
                    )

                    # ---- global pair ----
                    sgA = ps_st.tile([128, WIN], F32, tag="st")
                    nc.tensor.matmul(
                        sgA, gtT[0:64, hp, :], xTA, start=True, stop=True
                    )
                    sgB = ps_st.tile([128, WIN], F32, tag="st")
                    nc.tensor.matmul(
                        sgB, gtT[64:128, hp, :], xTB, start=True, stop=True
                    )
                    egA = expp.tile([128, WIN], F32R, tag="et")
                    nc.scalar.activation(out=egA, in_=sgA, func=ACT_EXP, scale=SCALE)
                    egB = expp.tile([128, WIN], F32R, tag="et")
                    nc.scalar.activation(out=egB, in_=sgB, func=ACT_EXP, scale=SCALE)
                    pgA = ps_pv.tile([HD + 1, WIN], F32, tag="pv")
                    nc.tensor.matmul(
                        pgA, gt_tok[:, 2 * hp, :], egA, start=True, stop=True
                    )
                    pgB = ps_pv.tile([HD + 1, WIN], F32, tag="pv")
                    nc.tensor.matmul(
                        pgB, gt_tok[:, 2 * hp + 1, :], egB, start=True, stop=True
                    )
                    normalize_pair(
                        pgA, pgB,
                        globT[0:64, hp, :], globT[64:128, hp, :],
                    )

                if dbg:
                    nc.gpsimd.dma_start(
                        out=dbg_l[w].rearrange("(db p) q -> p db q", p=128),
                        in_=localT,
                    )
                    nc.gpsimd.dma_start(
                        out=dbg_g[w].rearrange("(db p) q -> p db q", p=128),
                        in_=globT,
                    )

                if abl == "nogate":
                    for tt in range(4):
                        ost = outp.tile([128, D], F32, tag="ost")
                        nc.vector.tensor_copy(
                            out=ost.rearrange("p (a b) -> p a b", a=8),
                            in_=globT[:, :, tt * 128 : (tt + 1) * 128],
                        )
                        nc.sync.dma_start(out=out_r[w * 4 + tt], in_=ost)
                    continue

                # ---- gate: mixT[dout, t] = sigmoid(gw^T @ [localT;globT] + gb) ----
                mixT = mixp.tile([128, 8, WIN], BF16, tag="mx")
                for mt in range(8):
                    gp = ps_gate.tile([128, WIN], F32, tag="gate")
                    for kt in range(16):
                        rhs = localT[:, kt, :] if kt < 8 else globT[:, kt - 8, :]
                        nc.tensor.matmul(
                            gp,
                            gw_sb[:, kt, mt * 128 : (mt + 1) * 128],
                            rhs,
                            start=(kt == 0),
                            stop=(kt == 15),
                        )
                    nc.scalar.activation(
                        out=mixT[:, mt, :],
                        in_=gp,
                        func=ACT_SIG,
                        bias=gb_sb[:, mt : mt + 1],
                    )
                # comb in place AFTER every gate matmul has read localT/globT
                for mt in range(8):
                    nc.gpsimd.tensor_sub(
                        out=localT[:, mt, :], in0=localT[:, mt, :], in1=globT[:, mt, :]
                    )
                    nc.gpsimd.tensor_mul(
                        out=localT[:, mt, :], in0=localT[:, mt, :], in1=mixT[:, mt, :]
                    )
                    nc.gpsimd.tensor_add(
                        out=globT[:, mt, :], in0=globT[:, mt, :], in1=localT[:, mt, :]
                    )

                # ---- out projection: out[t, dout] = combT^T @ ow + ob ----
                for tt in range(4):
                    ost = outp.tile([128, D], F32, tag="ost")
                    for nt in range(2):
                        op = ps_big.tile([128, 512], F32, tag="big")
                        for kt in range(8):
                            nc.tensor.matmul(
                                op,
                                globT[:, kt, tt * 128 : (tt + 1) * 128],
                                ow_sb[:, kt, nt * 512 : (nt + 1) * 512],
                                start=(kt == 0),
                                stop=(kt == 7),
                            )
                        nc.vector.tensor_add(
                            out=ost[:, nt * 512 : (nt + 1) * 512],
                            in0=op,
                            in1=obb[:, nt * 512 : (nt + 1) * 512],
                        )
                    nc.sync.dma_start(out=out_r[w * 4 + tt], in_=ost)

        if nrep > 1:
            with tc.For_i(0, nrep, 1):
                emit_body()
        else:
            emit_body()

    return nc


# ---------------------------------------------------------------------------
# host side: sharding, compile-once runner, gather
# ---------------------------------------------------------------------------

IN_ORDER = ["x", "pos", "xg", "posg", "gmem", "cw", "cb", "gw", "gb", "ow", "ob"]


def shard_inputs(x, pos_emb, global_memory, conv_w, conv_b, gate_w, gate_b, out_w, out_b):
    x = np.asarray(x, np.float32)
    pos = np.asarray(pos_emb, np.float32)[:S]
    gmem = np.asarray(global_memory, np.float32)[0]
    cw = np.ascontiguousarray(
        np.asarray(conv_w, np.float32).transpose(2, 1, 0).reshape(4 * D, D)
    ).astype(BF16_NP)
    cb = np.asarray(conv_b, np.float32)[None, :]
    gw = np.asarray(gate_w, np.float32).astype(BF16_NP)
    gb = np.asarray(gate_b, np.float32).reshape(8, 128)
    ow = np.asarray(out_w, np.float32).astype(BF16_NP)
    ob = np.asarray(out_b, np.float32)[None, :]

    in_maps = []
    for c in range(N_CORES):
        b, off = c // 2, (c % 2) * SC
        in_maps.append(
            {
                "x": np.ascontiguousarray(x[b, off : off + SC]),
                "pos": np.ascontiguousarray(pos[off : off + SC]),
                "xg": np.ascontiguousarray(x[b, : 4 * G]),
                "posg": np.ascontiguousarray(pos[: 4 * G]),
                "gmem": gmem,
                "cw": cw,
                "cb": cb,
                "gw": gw,
                "gb": gb,
                "ow": ow,
                "ob": ob,
            }
        )
    return in_maps


class Runner:
    """Compile a Bass program once; execute repeatedly on 8 cores via PJRT."""

    def __init__(self, nc, n_cores=N_CORES):
        bass2jax.install_neuronx_cc_hook()
        split_sync_waits(nc)  # walrus-compat: <=1 sem wait per instruction
        self.nc = nc
        self.n_cores = n_cores
        partition_name = nc.partition_id_tensor.name if nc.partition_id_tensor else None
        in_names, out_names, out_avals = [], [], []
        for alloc in nc.m.functions[0].allocations:
            if not isinstance(alloc, mybir.MemoryLocationSet):
                continue
            name = alloc.memorylocations[0].name
            if alloc.kind == "ExternalInput":
                if name != partition_name:
                    in_names.append(name)
            elif alloc.kind == "ExternalOutput":
                out_names.append(name)
                out_avals.append(
                    jax.core.ShapedArray(
                        tuple(alloc.tensor_shape), mybir.dt.np(alloc.dtype)
                    )
                )
        self.in_names, self.out_names, self.out_avals = in_names, out_names, out_avals
        n_params = len(in_names)
        all_in_names = list(in_names) + list(out_names)
        if partition_name is not None:
            all_in_names.append(partition_name)

        def _body(*args):
            operands = list(args)
            if partition_name is not None:
                operands.append(bass2jax.partition_id_tensor())
            outs = bass2jax._bass_exec_p.bind(
                *operands,
                out_avals=tuple(out_avals),
                in_names=tuple(all_in_names),
                out_names=tuple(out_names),
                lowering_input_output_aliases=(),
                sim_require_finite=False,
                sim_require_nnan=False,
                nc=nc,
            )
            return tuple(outs)

        devices = jax.devices()[:n_cores]
        self.mesh = Mesh(np.asarray(devices), ("core",))
        self.sharded = jax.jit(
            shard_map(
                _body,
                mesh=self.mesh,
                in_specs=(PartitionSpec("core"),) * (n_params + len(out_names)),
                out_specs=(PartitionSpec("core"),) * len(out_names),
                check_rep=False,
            )
        )

    def put(self, in_maps):
        concat = [
            np.concatenate(
                [np.asarray(in_maps[c][n]) for c in range(self.n_cores)], axis=0
            )
            for n in self.in_names
        ]
        for av in self.out_avals:
            concat.append(
                np.zeros((self.n_cores * av.shape[0], *av.shape[1:]), av.dtype)
            )
        return [jax.device_put(a) for a in concat]

    def run(self, dev_args):
        outs = self.sharded(*dev_args)
        jax.block_until_ready(outs)
        return outs

    def get(self, outs):
        res = []
        for c in range(self.n_cores):
            d = {}
            for i, n in enumerate(self.out_names):
                d[n] = np.asarray(outs[i]).reshape(
                    self.n_cores, *self.out_avals[i].shape
                )[c]
            res.append(d)
        return res


_RUNNERS: dict = {}


def get_runner(nrep: int = 1) -> Runner:
    if nrep not in _RUNNERS:
        _RUNNERS[nrep] = Runner(build_nc(nrep))
    return _RUNNERS[nrep]


def kernel(**inputs) -> np.ndarray:
    in_maps = shard_inputs(**inputs)
    runner = get_runner(1)
    res = runner.get(runner.run(runner.put(in_maps)))
    out = np.empty((B, S, D), np.float32)
    for c in range(N_CORES):
        b, off = c // 2, (c % 2) * SC
        out[b, off : off + SC] = res[c]["out"]
    return out



# revision 10
# speedup vs baseline: 1.1468x; 1.1468x over previous
"""Trainium2 Bass kernel for sparse_attention nn_CWAB_34050500722860.

Model (per batch b, S=4096 tokens, D=1024, H=16 heads of 64):
  xp = x + pos_emb[:S]
  local: non-overlapping 512-token window self-attention per head
  global: keys = concat(conv4-compressed first-256-tokens (G=64), global_memory (64))
  out = sigmoid([local||glob] @ gate_w + gate_b) gated mix, then @ out_w + out_b

Sharding: 8 cores = 4 batches x 2 sequence halves (2048 tokens = 4 windows
per core). Every core computes its batch's 128 global tokens from a
replicated copy of the first 256 tokens, so there are no collectives.

Dataflow (per core, all matmuls on PE):
  - scores are computed TRANSPOSED: sT[k,q] = lhsT(xT k-slice)^T @ rhs(xT q)
    so softmax normalization runs over the PSUM partition dim and the
    attention output comes out feature-major (localT[d,q]) - exactly the
    layout the gate matmul needs as lhsT. No probability transposes.
  - head pair A/B score matmuls write the two halves of one 2-bank PSUM
    tile (row-tiled 64x128 PE mode, concurrent); ONE exp activation
    [128,1024] covers both.
  - V carries an appended ones column, so the PV matmul's row 64 is the
    softmax denominator (colsum) for free. Normalization: the two Z rows
    are broadcast across partitions by K=1 selector matmuls, reciprocated
    in one [128,512] reciprocal_approx_fast, and applied in the PSUM->SBUF
    evacuation multiply.
  - max-subtraction is skipped: scores*scale is ~N(0,1) with worst case
    ~14 (self-attention diagonal), exp() stays far inside fp32 range.
  - the sigmoid gate runs as tanh (same ACT table set as exp - no table
    thrash): sigmoid(x) = (1+tanh(x/2))/2, with gate_b and out_w pre-halved
    on the host. comb = (t+1)*local - (t-1)*glob, then @ (out_w/2).
  - attention path float32r (TF32-class); gate/out/conv matmuls bf16.
"""

import os
from contextlib import ExitStack

import numpy as np
import ml_dtypes

import jax
import jax.numpy as jnp
from jax.sharding import Mesh, PartitionSpec
from jax.experimental.shard_map import shard_map

import bass_rust
import concourse.bass as bass
import concourse.mybir as mybir
import concourse.tile as tile
from concourse import bass2jax
from concourse.masks import make_identity
from concourse.vector_clock import ScopedClock

F32 = mybir.dt.float32
F32R = mybir.dt.float32r
BF16 = mybir.dt.bfloat16
BF16_NP = ml_dtypes.bfloat16

N_CORES = 8
B, S, D = 4, 4096, 1024
H, HD, WIN, G = 16, 64, 512, 64
SC = S // 2          # tokens per core
NW = SC // WIN       # 4 windows per core
SCALE = 1.0 / np.sqrt(HD)
ACT_EXP = mybir.ActivationFunctionType.Exp
ACT_LN = mybir.ActivationFunctionType.Ln
ALU_ADD = mybir.AluOpType.add
ALU_SUB = mybir.AluOpType.subtract
ALU_MULT = mybir.AluOpType.mult


class SplitDrainTileContext(tile.TileContext):
    """Walrus in this container rejects a Drain carrying >2 sem waits.

    Tile's exit puts a wait for every engine/queue sem on one Drain; strip
    them and re-emit as single-wait NOPs on the sync sequencer (sequencer
    order keeps the all-engine barrier behind every wait)."""

    def _drain_and_barrier(self, tick_clock, wait_clock):
        nc = self.nc
        drain_inst = nc.sync.drain()
        wait_clock.add_sem_waits(
            drain_inst.ins, ScopedClock({None: tick_clock.global_clock})
        )
        si = drain_inst.ins.sync_info
        waits = list(si.on_wait) if si is not None and si.on_wait else []
        if len(waits) > 1:
            updates = list(si.on_update) if si is not None and si.on_update else []
            drain_inst.ins.sync_info = bass_rust.SyncInfo(
                on_wait=waits[:1], on_update=updates
            )
            for w in waits[1:]:
                nop = nc.sync.nop()
                nop.ins.sync_info = bass_rust.SyncInfo(on_wait=[w], on_update=[])

        nc.all_engine_barrier()
        assert self.sems is not None
        popped = nc._tile_sem_poison_stack.pop()
        assert popped is self._sem_poison
        nc.clear_and_free_semaphores(list(self.sems.allocated().values()))
        nc.all_engine_barrier()


def split_sync_waits(nc, limit: int = 1):
    """This walrus build accepts at most one sem wait per instruction.

    For any instruction carrying more, peel the extras onto same-engine
    NoOps inserted directly before it (engine streams preserve bb order,
    so the engine still blocks on every wait before executing it)."""
    for fn in nc.m.functions:
        for bb in fn.blocks:
            il = bb.instructions
            out, changed, k = [], False, 0
            for inst in il:
                si = inst.sync_info
                waits = list(si.on_wait) if si is not None and si.on_wait else []
                if len(waits) > limit and inst.engine != mybir.EngineType.Unassigned:
                    for w in waits[:-limit]:
                        nop = mybir.InstNoOp(
                            name=f"{inst.name}-wsplit{k}", engine=inst.engine
                        )
                        nop.sync_info = bass_rust.SyncInfo(on_wait=[w], on_update=[])
                        out.append(nop)
                        k += 1
                    inst.sync_info = bass_rust.SyncInfo(
                        on_wait=waits[-limit:],
                        on_update=list(si.on_update) if si.on_update else [],
                    )
                    changed = True
                out.append(inst)
            if changed:
                bb.instructions = out


def build_nc(nrep: int = 1):
    nc = bass.Bass("TRN2", target_bir_lowering=False, debug=False)

    x_d = nc.declare_dram_parameter("x", [SC, D], F32, isOutput=False).ap()
    pos_d = nc.declare_dram_parameter("pos", [SC, D], F32, isOutput=False).ap()
    xg_d = nc.declare_dram_parameter("xg", [4 * G, D], F32, isOutput=False).ap()
    posg_d = nc.declare_dram_parameter("posg", [4 * G, D], F32, isOutput=False).ap()
    gmem_d = nc.declare_dram_parameter("gmem", [G, D], F32, isOutput=False).ap()
    cw_d = nc.declare_dram_parameter("cw", [4 * D, D], BF16, isOutput=False).ap()
    cb_d = nc.declare_dram_parameter("cb", [1, D], F32, isOutput=False).ap()
    gw_d = nc.declare_dram_parameter("gw", [2 * D, D], BF16, isOutput=False).ap()
    gb_d = nc.declare_dram_parameter("gb", [8, 128], F32, isOutput=False).ap()
    ow_d = nc.declare_dram_parameter("ow", [D, D], BF16, isOutput=False).ap()
    ob_d = nc.declare_dram_parameter("ob", [1, D], F32, isOutput=False).ap()
    out_d = nc.declare_dram_parameter("out", [SC, D], F32, isOutput=True).ap()

    x_r = x_d.rearrange("(n p) d -> n p d", p=128)       # [16,128,1024]
    pos_r = pos_d.rearrange("(n p) d -> n p d", p=128)
    xg_r = xg_d.rearrange("(n p) d -> n p d", p=128)     # [2,128,1024]
    posg_r = posg_d.rearrange("(n p) d -> n p d", p=128)
    out_r = out_d.rearrange("(n p) d -> n p d", p=128)

    with SplitDrainTileContext(nc) as tc, ExitStack() as ctx:
        const = ctx.enter_context(tc.tile_pool(name="const", bufs=1))
        wpool = ctx.enter_context(tc.tile_pool(name="weights", bufs=1))
        cwp = ctx.enter_context(tc.tile_pool(name="cw_stream", bufs=3))
        stage = ctx.enter_context(tc.tile_pool(name="stage", bufs=2))
        xwp = ctx.enter_context(tc.tile_pool(name="xw", bufs=2))
        xtp = ctx.enter_context(tc.tile_pool(name="xt", bufs=2))
        expp = ctx.enter_context(tc.tile_pool(name="expt", bufs=3))
        zrp = ctx.enter_context(tc.tile_pool(name="zrow", bufs=1))
        rbp = ctx.enter_context(tc.tile_pool(name="rb", bufs=2))
        ltp = ctx.enter_context(tc.tile_pool(name="localT", bufs=2))
        gtp = ctx.enter_context(tc.tile_pool(name="globT", bufs=2))
        mixp = ctx.enter_context(tc.tile_pool(name="mix", bufs=1))
        outp = ctx.enter_context(tc.tile_pool(name="ostage", bufs=1))
        gpool = ctx.enter_context(tc.tile_pool(name="gt", bufs=1))

        ps_tr = ctx.enter_context(tc.tile_pool(name="ps_tr", bufs=1, space="PSUM"))
        ps_st = ctx.enter_context(tc.tile_pool(name="ps_st", bufs=2, space="PSUM"))
        ps_pv = ctx.enter_context(tc.tile_pool(name="ps_pv", bufs=2, space="PSUM"))
        ps_sh = ctx.enter_context(tc.tile_pool(name="ps_sh", bufs=1, space="PSUM"))

        # ---- constants / persistent weights (outside the repeat loop) ----
        ident_f = const.tile([128, 128], F32)
        make_identity(nc, ident_f)
        ident_b = const.tile([128, 128], BF16)
        nc.vector.tensor_copy(out=ident_b, in_=ident_f)
        ones_f = const.tile([128, 64], F32)
        nc.vector.memset(ones_f, 1.0)
        # selectors (K=1 lhsT rows): top covers out partitions 0-63, bottom
        # 64-127; two accumulating matmuls replicate a head-pair's denominator
        # rows into one [128, q] psum bank.
        selt_f = const.tile([1, 128], F32)
        nc.vector.memset(selt_f, 0.0)
        nc.vector.memset(selt_f[0:1, 0:64], 1.0)
        selb_f = const.tile([1, 128], F32)
        nc.vector.memset(selb_f, 0.0)
        nc.vector.memset(selb_f[0:1, 64:128], 1.0)
        sel_top = const.tile([1, 128], F32R)
        nc.vector.tensor_copy(out=sel_top, in_=selt_f)
        sel_bot = const.tile([1, 128], F32R)
        nc.vector.tensor_copy(out=sel_bot, in_=selb_f)

        gw_sb = wpool.tile([128, 16, D], BF16)
        nc.sync.dma_start(out=gw_sb, in_=gw_d.rearrange("(k p) n -> p k n", p=128))
        ow_sb = wpool.tile([128, 8, D], BF16)
        nc.sync.dma_start(out=ow_sb, in_=ow_d.rearrange("(k p) n -> p k n", p=128))
        gb_sb = wpool.tile([128, 8], F32)
        nc.sync.dma_start(out=gb_sb, in_=gb_d.rearrange("m p -> p m"))
        obb = wpool.tile([128, D], F32)
        nc.sync.dma_start(out=obb, in_=ob_d.to_broadcast([128, D]))
        cbb = wpool.tile([G, D], F32)
        nc.sync.dma_start(out=cbb, in_=cb_d.to_broadcast([G, D]))
        gm_sb = wpool.tile([G, D], F32)
        nc.sync.dma_start(out=gm_sb, in_=gmem_d)

        def normalize_pair(pvA, pvB, outA, outB):
            """softmax denominators sit in row HD of pvA/pvB; write the
            normalized [64, 512] blocks to outA/outB (bf16 SBUF)."""
            zrow = zrp.tile([1, 2, WIN], F32R, tag="zr")
            nc.vector.tensor_copy(out=zrow[:, 0, :], in_=pvA[HD : HD + 1, :])
            nc.vector.tensor_copy(out=zrow[:, 1, :], in_=pvB[HD : HD + 1, :])
            rcb = ps_st.tile([128, 2, WIN], F32, tag="st")
            nc.tensor.matmul(
                rcb[:, 0, :], sel_top, zrow[:, 0, :], start=True, stop=False
            )
            nc.tensor.matmul(
                rcb[:, 0, :], sel_bot, zrow[:, 1, :], start=False, stop=True
            )
            zl = rbp.tile([128, WIN], F32, tag="zl")
            nc.scalar.activation(out=zl, in_=rcb[:, 0, :], func=ACT_LN)
            rb = rbp.tile([128, WIN], F32, tag="rb")
            nc.scalar.activation(out=rb, in_=zl, func=ACT_EXP, scale=-1.0)
            nc.vector.tensor_mul(out=outA, in0=pvA[0:HD, :], in1=rb[0:64, :])
            nc.vector.tensor_mul(out=outB, in0=pvB[0:HD, :], in1=rb[64:128, :])

        def emit_body():
            # ================= conv -> gt (global tokens) =================
            # xg_sum = bf16(xg + posg), token-major [128, 2, 1024]
            xg_sum = gpool.tile([128, 2, D], BF16, tag="xgsum")
            for tt in range(2):
                for hh in range(2):
                    xs = stage.tile([128, 512], F32, tag="xs")
                    nc.sync.dma_start(
                        out=xs, in_=xg_r[tt][:, hh * 512 : (hh + 1) * 512]
                    )
                    ps = stage.tile([128, 512], F32, tag="ps")
                    nc.sync.dma_start(
                        out=ps, in_=posg_r[tt][:, hh * 512 : (hh + 1) * 512]
                    )
                    nc.gpsimd.tensor_add(
                        out=xg_sum[:, tt, hh * 512 : (hh + 1) * 512], in0=xs, in1=ps
                    )

            # xgp[p, k, ib, g] = xg_sum[4g+k, ib*128+p]  (bf16 lhsT for conv)
            # xgp[p, k, ib, g] = xg_sum[4g+k, ib*128+p]  (bf16 lhsT for conv)
            # transposed in [128tok, 64ch] chunks: slot (tt, sub) holds
            # channels ib*128+sub*64.. as partitions, tokens tt*128.. as free.
            xgp = gpool.tile([128, 4, 8, G], BF16, tag="xgp")
            for ib in range(8):
                pt = ps_tr.tile([64, 4, 128], BF16, tag="tr")
                for tt in range(2):
                    for sub in range(2):
                        nc.tensor.transpose(
                            pt[:, 2 * tt + sub, :],
                            xg_sum[:, tt, ib * 128 + sub * 64 : ib * 128 + sub * 64 + 64],
                            ident_b,
                        )
                for sub in range(2):
                    for k in range(4):
                        nc.vector.tensor_copy(
                            out=xgp[
                                sub * 64 : sub * 64 + 64, k, ib, :
                            ].rearrange("p (a b) -> p a b", a=2),
                            in_=pt[:, sub :: 2, k :: 4],
                        )

            # comp[g, o] = sum_f xgT[f, g] cw[f, o] + cb; both 512-col halves
            # run as concurrent col-tiles (M=64 each) of ONE psum bank.
            gt_tok = gpool.tile([128, H, HD + 1], BF16, tag="gt_tok")
            comp = ps_sh.tile([128, 512], F32, tag="sh")
            for kt in range(32):
                cwc = cwp.tile([128, D], BF16, tag="cw")
                nc.sync.dma_start(
                    out=cwc, in_=cw_d.rearrange("(k p) n -> k p n", p=128)[kt]
                )
                k, ib = kt // 8, kt % 8
                nc.tensor.matmul(
                    comp[0:G, :], xgp[:, k, ib, :], cwc[:, 0:512],
                    start=(kt == 0), stop=(kt == 31),
                )
                nc.tensor.matmul(
                    comp[G:128, :], xgp[:, k, ib, :], cwc[:, 512:1024],
                    start=(kt == 0), stop=(kt == 31),
                )
            for nt in range(2):
                nc.vector.tensor_add(
                    out=gt_tok[0:G, nt * 8 : (nt + 1) * 8, 0:HD],
                    in0=comp[nt * G : (nt + 1) * G, :].rearrange(
                        "p (h d) -> p h d", h=8
                    ),
                    in1=cbb[:, nt * 512 : (nt + 1) * 512].rearrange(
                        "p (h d) -> p h d", h=8
                    ),
                )
            nc.vector.tensor_copy(
                out=gt_tok[G:128, :, 0:HD],
                in_=gm_sb.rearrange("g (h d) -> g h d", h=H),
            )
            nc.vector.tensor_copy(out=gt_tok[:, :, HD], in_=ones_f[:, 0:H])

            # gtT[p, hp, g] = gt[g, hp*128+p]
            gtT = gpool.tile([128, 8, 128], BF16, tag="gtT")
            for hp in range(8):
                pt = ps_tr.tile([64, 4, 128], BF16, tag="tr")
                for sub in range(2):
                    nc.tensor.transpose(
                        pt[:, sub, :], gt_tok[:, 2 * hp + sub, 0:HD], ident_b
                    )
                for sub in range(2):
                    nc.vector.tensor_copy(
                        out=gtT[sub * 64 : sub * 64 + 64, hp, :],
                        in_=pt[:, sub, :],
                    )

            # ========================= windows =========================
            for w in range(NW):
                # xw[p, kt, h, 0:64] = f32r(x+pos), col 64 = 1  (V_aug, PV lhsT)
                xw = xwp.tile([128, 4, H, HD + 1], BF16, tag="xw")
                for kt in range(4):
                    for hh in range(2):
                        xs = stage.tile([128, 512], F32, tag="xs")
                        nc.sync.dma_start(
                            out=xs, in_=x_r[w * 4 + kt][:, hh * 512 : (hh + 1) * 512]
                        )
                        ps = stage.tile([128, 512], F32, tag="ps")
                        nc.sync.dma_start(
                            out=ps, in_=pos_r[w * 4 + kt][:, hh * 512 : (hh + 1) * 512]
                        )
                        nc.gpsimd.tensor_add(
                            out=xw[:, kt, hh * 8 : (hh + 1) * 8, 0:HD],
                            in0=xs.rearrange("p (h d) -> p h d", h=8),
                            in1=ps.rearrange("p (h d) -> p h d", h=8),
                        )
                nc.gpsimd.tensor_copy(
                    out=xw[:, :, :, HD],
                    in_=ones_f.rearrange("p (a b) -> p a b", a=4),
                )

                # xT[p, hp, q] = xp[w*512+q, hp*128+p]; one [128,128] PE
                # transpose per (hp, kt) covers the head pair (65-stride AP).
                xT = xtp.tile([128, 8, WIN], BF16, tag="xT")
                for hp in range(8):
                    for sub in range(2):
                        pt = ps_tr.tile([64, 4, 128], BF16, tag="tr")
                        for kt in range(4):
                            nc.tensor.transpose(
                                pt[:, kt, :], xw[:, kt, 2 * hp + sub, 0:HD], ident_b
                            )
                        nc.vector.tensor_copy(
                            out=xT[sub * 64 : sub * 64 + 64, hp, :],
                            in_=pt.rearrange("p a b -> p (a b)"),
                        )

                localT = ltp.tile([128, 8, WIN], BF16, tag="localT")
                globT = gtp.tile([128, 8, WIN], BF16, tag="globT")

                for hp in range(8):
                    xTA = xT[0:64, hp, :]      # head 2hp   [64, 512]
                    xTB = xT[64:128, hp, :]    # head 2hp+1 [64, 512]

                    # ---- local pair: sT pairs, exp, PV(ones-col), normalize
                    pvA = ps_pv.tile([HD + 1, WIN], F32, tag="pv")
                    pvB = ps_pv.tile([HD + 1, WIN], F32, tag="pv")
                    for ktp in range(2):
                        sts = []
                        ets = []
                        for kk in range(2):
                            kt = 2 * ktp + kk
                            st = ps_st.tile([128, 2, WIN], F32, tag="st")
                            nc.tensor.matmul(
                                st[:, 0, :],
                                xTA[:, kt * 128 : (kt + 1) * 128], xTA,
                                start=True, stop=True,
                            )
                            nc.tensor.matmul(
                                st[:, 1, :],
                                xTB[:, kt * 128 : (kt + 1) * 128], xTB,
                                start=True, stop=True,
                            )
                            sts.append(st)
                        for kk in range(2):
                            et = expp.tile([128, 2, WIN], BF16, tag="et")
                            nc.scalar.activation(
                                out=et.rearrange("p a q -> p (a q)"),
                                in_=sts[kk].rearrange("p a q -> p (a q)"),
                                func=ACT_EXP, scale=SCALE,
                            )
                            ets.append(et)
                        for kk in range(2):
                            kt = 2 * ktp + kk
                            nc.tensor.matmul(
                                pvA, xw[:, kt, 2 * hp, :], ets[kk][:, 0, :],
                                start=(kt == 0), stop=(kt == 3),
                            )
                            nc.tensor.matmul(
                                pvB, xw[:, kt, 2 * hp + 1, :], ets[kk][:, 1, :],
                                start=(kt == 0), stop=(kt == 3),
                            )
                    normalize_pair(
                        pvA, pvB,
                        localT[0:64, hp, :], localT[64:128, hp, :],
                    )

                    # ---- global pair ----
                    sg = ps_st.tile([128, 2, WIN], F32, tag="st")
                    nc.tensor.matmul(
                        sg[:, 0, :], gtT[0:64, hp, :], xTA, start=True, stop=True
                    )
                    nc.tensor.matmul(
                        sg[:, 1, :], gtT[64:128, hp, :], xTB, start=True, stop=True
                    )
                    eg = expp.tile([128, 2, WIN], BF16, tag="et")
                    nc.scalar.activation(
                        out=eg.rearrange("p a q -> p (a q)"),
                        in_=sg.rearrange("p a q -> p (a q)"),
                        func=ACT_EXP, scale=SCALE,
                    )
                    pgA = ps_pv.tile([HD + 1, WIN], F32, tag="pv")
                    nc.tensor.matmul(
                        pgA, gt_tok[:, 2 * hp, :], eg[:, 0, :], start=True, stop=True
                    )
                    pgB = ps_pv.tile([HD + 1, WIN], F32, tag="pv")
                    nc.tensor.matmul(
                        pgB, gt_tok[:, 2 * hp + 1, :], eg[:, 1, :],
                        start=True, stop=True,
                    )
                    normalize_pair(
                        pgA, pgB,
                        globT[0:64, hp, :], globT[64:128, hp, :],
                    )

                # ---- gate: mix = sigmoid(gw^T @ [localT;globT] + gb),
                # computed as exp(-ln(1 + exp(-x))) so every activation in the
                # kernel shares the natural_log_exp ACT table set (gb negated
                # on host and applied as the pass-1 bias).
                mixT = mixp.tile([128, 8, WIN], BF16, tag="mx")
                for mt in range(8):
                    gp = ps_sh.tile([128, WIN], F32, tag="sh")
                    for kt in range(16):
                        rhs = localT[:, kt, :] if kt < 8 else globT[:, kt - 8, :]
                        nc.tensor.matmul(
                            gp,
                            gw_sb[:, kt, mt * 128 : (mt + 1) * 128],
                            rhs,
                            start=(kt == 0),
                            stop=(kt == 15),
                        )
                    s1 = rbp.tile([128, WIN], F32, tag="zl")
                    nc.scalar.activation(
                        out=s1, in_=gp, func=ACT_EXP, scale=-1.0,
                        bias=gb_sb[:, mt : mt + 1],
                    )
                    s2 = rbp.tile([128, WIN], F32, tag="rb")
                    nc.scalar.activation(out=s2, in_=s1, func=ACT_LN, bias=1.0)
                    nc.scalar.activation(
                        out=mixT[:, mt, :], in_=s2, func=ACT_EXP, scale=-1.0
                    )
                # comb in place AFTER every gate matmul has read localT/globT
                for mt in range(8):
                    nc.gpsimd.tensor_sub(
                        out=localT[:, mt, :], in0=localT[:, mt, :],
                        in1=globT[:, mt, :],
                    )
                    nc.gpsimd.tensor_mul(
                        out=localT[:, mt, :], in0=localT[:, mt, :],
                        in1=mixT[:, mt, :],
                    )
                    nc.gpsimd.tensor_add(
                        out=globT[:, mt, :], in0=globT[:, mt, :],
                        in1=localT[:, mt, :],
                    )

                # ---- out projection: out[t, dout] = combT^T @ ow + ob ----
                for tt in range(4):
                    ost = outp.tile([128, D], F32, tag="ost")
                    for nt in range(2):
                        op = ps_sh.tile([128, 512], F32, tag="sh")
                        for kt in range(8):
                            nc.tensor.matmul(
                                op,
                                globT[:, kt, tt * 128 : (tt + 1) * 128],
                                ow_sb[:, kt, nt * 512 : (nt + 1) * 512],
                                start=(kt == 0),
                                stop=(kt == 7),
                            )
                        nc.vector.tensor_add(
                            out=ost[:, nt * 512 : (nt + 1) * 512],
                            in0=op,
                            in1=obb[:, nt * 512 : (nt + 1) * 512],
                        )
                    nc.sync.dma_start(out=out_r[w * 4 + tt], in_=ost)

        if nrep > 1:
            with tc.For_i(0, nrep, 1):
                emit_body()
        else:
            emit_body()

    return nc


# ---------------------------------------------------------------------------
# host side: sharding, compile-once runner, gather
# ---------------------------------------------------------------------------

IN_ORDER = ["x", "pos", "xg", "posg", "gmem", "cw", "cb", "gw", "gb", "ow", "ob"]


def shard_inputs(x, pos_emb, global_memory, conv_w, conv_b, gate_w, gate_b, out_w, out_b):
    x = np.asarray(x, np.float32)
    pos = np.asarray(pos_emb, np.float32)[:S]
    gmem = np.asarray(global_memory, np.float32)[0]
    cw = np.ascontiguousarray(
        np.asarray(conv_w, np.float32).transpose(2, 1, 0).reshape(4 * D, D)
    ).astype(BF16_NP)
    cb = np.asarray(conv_b, np.float32)[None, :]
    gw = np.asarray(gate_w, np.float32).astype(BF16_NP)
    # sigmoid computed as exp(-ln(1+exp(-(x+gb)))) on device -> gb negated
    # so it can ride the pass-1 activation bias.
    gb = (-np.asarray(gate_b, np.float32)).reshape(8, 128)
    ow = np.asarray(out_w, np.float32).astype(BF16_NP)
    ob = np.asarray(out_b, np.float32)[None, :]

    in_maps = []
    for c in range(N_CORES):
        b, off = c // 2, (c % 2) * SC
        in_maps.append(
            {
                "x": np.ascontiguousarray(x[b, off : off + SC]),
                "pos": np.ascontiguousarray(pos[off : off + SC]),
                "xg": np.ascontiguousarray(x[b, : 4 * G]),
                "posg": np.ascontiguousarray(pos[: 4 * G]),
                "gmem": gmem,
                "cw": cw,
                "cb": cb,
                "gw": gw,
                "gb": gb,
                "ow": ow,
                "ob": ob,
            }
        )
    return in_maps


class Runner:
    """Compile a Bass program once; execute repeatedly on 8 cores via PJRT."""

    def __init__(self, nc, n_cores=N_CORES):
        bass2jax.install_neuronx_cc_hook()
        split_sync_waits(nc)  # walrus-compat: <=1 sem wait per instruction
        self.nc = nc
        self.n_cores = n_cores
        partition_name = nc.partition_id_tensor.name if nc.partition_id_tensor else None
        in_names, out_names, out_avals = [], [], []
        for alloc in nc.m.functions[0].allocations:
            if not isinstance(alloc, mybir.MemoryLocationSet):
                continue
            name = alloc.memorylocations[0].name
            if alloc.kind == "ExternalInput":
                if name != partition_name:
                    in_names.append(name)
            elif alloc.kind == "ExternalOutput":
                out_names.append(name)
                out_avals.append(
                    jax.core.ShapedArray(
                        tuple(alloc.tensor_shape), mybir.dt.np(alloc.dtype)
                    )
                )
        self.in_names, self.out_names, self.out_avals = in_names, out_names, out_avals
        n_params = len(in_names)
        all_in_names = list(in_names) + list(out_names)
        if partition_name is not None:
            all_in_names.append(partition_name)

        def _body(*args):
            operands = list(args)
            if partition_name is not None:
                operands.append(bass2jax.partition_id_tensor())
            outs = bass2jax._bass_exec_p.bind(
                *operands,
                out_avals=tuple(out_avals),
                in_names=tuple(all_in_names),
                out_names=tuple(out_names),
                lowering_input_output_aliases=(),
                sim_require_finite=False,
                sim_require_nnan=False,
                nc=nc,
            )
            return tuple(outs)

        devices = jax.devices()[:n_cores]
        self.mesh = Mesh(np.asarray(devices), ("core",))
        self.sharded = jax.jit(
            shard_map(
                _body,
                mesh=self.mesh,
                in_specs=(PartitionSpec("core"),) * (n_params + len(out_names)),
                out_specs=(PartitionSpec("core"),) * len(out_names),
                check_rep=False,
            )
        )

    def put(self, in_maps):
        concat = [
            np.concatenate(
                [np.asarray(in_maps[c][n]) for c in range(self.n_cores)], axis=0
            )
            for n in self.in_names
        ]
        for av in self.out_avals:
            concat.append(
                np.zeros((self.n_cores * av.shape[0], *av.shape[1:]), av.dtype)
            )
        return [jax.device_put(a) for a in concat]

    def run(self, dev_args):
        outs = self.sharded(*dev_args)
        jax.block_until_ready(outs)
        return outs

    def get(self, outs):
        res = []
        for c in range(self.n_cores):
            d = {}
            for i, n in enumerate(self.out_names):
                d[n] = np.asarray(outs[i]).reshape(
                    self.n_cores, *self.out_avals[i].shape
                )[c]
            res.append(d)
        return res


_RUNNERS: dict = {}


def get_runner(nrep: int = 1) -> Runner:
    if nrep not in _RUNNERS:
        _RUNNERS[nrep] = Runner(build_nc(nrep))
    return _RUNNERS[nrep]


def kernel(**inputs) -> np.ndarray:
    in_maps = shard_inputs(**inputs)
    runner = get_runner(1)
    res = runner.get(runner.run(runner.put(in_maps)))
    out = np.empty((B, S, D), np.float32)
    for c in range(N_CORES):
        b, off = c // 2, (c % 2) * SC
        out[b, off : off + SC] = res[c]["out"]
    return out
